# revision 1
# baseline (speedup 1.0000x reference)
"""Trainium2 Bass kernel for an AttentionBlock (GroupNorm + single-head
self-attention over spatial positions + residual).

Reference computation (B=32, C=512, H=W=32, N=H*W=1024):
    xn = GroupNorm(32 groups)(x) * gamma + beta
    q/k/v = W{q,k,v} @ xn + b         (per batch, [C, N])
    score = q^T k / sqrt(C)           ([N, N])
    attn  = softmax(score, axis=-1)
    out   = Wo @ (v @ attn^T) + bo    ([C, N])
    y     = out + xn

Sharding: data-parallel over batch across 8 NeuronCores (4 batches each);
weights replicated. GroupNorm statistics, softmax and the residual run in
fp32; matmul operands are bf16 (fp32 PSUM accumulation), giving ~4e-4
relative error on the full output.
"""

import os
import sys

for _p in ("/opt/trn_rl_repo", "/root/.axon_site/_ro/trn_rl_repo"):
    if os.path.isdir(_p) and _p not in sys.path:
        sys.path.insert(0, _p)

import numpy as np
import ml_dtypes

import concourse.bass as bass
import concourse.mybir as mybir
import concourse.tile as tile
from concourse import bacc
from concourse.bass_utils import run_bass_kernel_spmd

# Problem constants (hardcoded per harness contract)
B, C, HH, WW = 32, 512, 32, 32
HW = HH * WW                  # 1024 sequence positions
NCORES = 8
BL = B // NCORES              # batches per core
G = 32                        # groups
GS = C // G                   # channels per group (16)
P = 128                       # partitions
CT = C // P                   # channel chunks (4)
NT = HW // P                  # sequence chunks (8)
NHALF = HW // 512             # 512-wide free-dim halves (2)
EPS = 1e-5
SCALE = float(C) ** -0.5
F32 = mybir.dt.float32
BF16 = mybir.dt.bfloat16
AF = mybir.ActivationFunctionType
ALU = mybir.AluOpType


def _host_constants():
    # gmat[p, t, g] = 1/(16*HW) if channel (t*128+p) is in group g
    gmat = np.zeros((P, CT, G), dtype=np.float32)
    # hmat[g, t, p] = 1 if channel (t*128+p) is in group g (group -> channel)
    hmat = np.zeros((P, CT, P), dtype=np.float32)
    for t in range(CT):
        for p in range(P):
            g = (t * P + p) // GS
            gmat[p, t, g] = 1.0 / (GS * HW)
            hmat[g, t, p] = 1.0
    ident = np.eye(P, dtype=ml_dtypes.bfloat16)
    return gmat, hmat, ident


def build_module():
    nc = bacc.Bacc("TRN2", target_bir_lowering=False, debug=False)

    x = nc.dram_tensor("x", [BL, C, HW], F32, kind="ExternalInput").ap()
    y = nc.dram_tensor("y", [BL, C, HW], F32, kind="ExternalOutput").ap()
    wqT = nc.dram_tensor("wqT", [C, C], BF16, kind="ExternalInput").ap()
    wkT = nc.dram_tensor("wkT", [C, C], BF16, kind="ExternalInput").ap()
    wvT = nc.dram_tensor("wvT", [C, C], BF16, kind="ExternalInput").ap()
    woT = nc.dram_tensor("woT", [C, C], BF16, kind="ExternalInput").ap()
    gamma = nc.dram_tensor("gamma", [C], F32, kind="ExternalInput").ap()
    beta = nc.dram_tensor("beta", [C], F32, kind="ExternalInput").ap()
    bq = nc.dram_tensor("bq", [C], F32, kind="ExternalInput").ap()
    bk = nc.dram_tensor("bk", [C], F32, kind="ExternalInput").ap()
    bv = nc.dram_tensor("bv", [C], F32, kind="ExternalInput").ap()
    bo = nc.dram_tensor("bo", [C], F32, kind="ExternalInput").ap()
    gmat = nc.dram_tensor("gmat", [P, CT, G], F32, kind="ExternalInput").ap()
    hmat = nc.dram_tensor("hmat", [P, CT, P], F32, kind="ExternalInput").ap()
    ident = nc.dram_tensor("ident", [P, P], BF16, kind="ExternalInput").ap()

    def pc(v):  # [C] dram -> [P, CT] sbuf layout (channel c = t*128+p)
        return v.rearrange("(t p) -> p t", p=P)

    with tile.TileContext(nc) as tc:
        with (
            tc.tile_pool(name="singles", bufs=1) as singles,
            tc.tile_pool(name="xpool", bufs=2) as xpool,
            tc.tile_pool(name="acts", bufs=2) as acts,
            tc.tile_pool(name="ypool", bufs=1) as ypool,
            tc.tile_pool(name="attn", bufs=3) as attnp,
            tc.tile_pool(name="erows", bufs=8) as erows,
            tc.tile_pool(name="small", bufs=4) as small,
            tc.tile_pool(name="pmm", bufs=7, space="PSUM") as pmm,
            tc.tile_pool(name="pst", bufs=1, space="PSUM") as pst,
        ):
            # ---- batch-0 input first: its stats chain is the critical path ----
            xs_tiles = []
            xs0 = xpool.tile([P, CT, HW], F32, tag="xs")
            xs_tiles.append(xs0)
            x0r = x[0].rearrange("(t p) n -> p t n", p=P)
            for t in range(CT):
                nc.sync.dma_start(out=xs0[:, t, :], in_=x0r[:, t, :])

            # ---- load constants / weights once ----
            wq_s = singles.tile([P, CT, C], BF16)
            wk_s = singles.tile([P, CT, C], BF16)
            wv_s = singles.tile([P, CT, C], BF16)
            wo_s = singles.tile([P, CT, C], BF16)
            nc.sync.dma_start(out=wq_s, in_=wqT.rearrange("(t p) o -> p t o", p=P))
            nc.sync.dma_start(out=wk_s, in_=wkT.rearrange("(t p) o -> p t o", p=P))
            nc.sync.dma_start(out=wv_s, in_=wvT.rearrange("(t p) o -> p t o", p=P))
            nc.sync.dma_start(out=wo_s, in_=woT.rearrange("(t p) o -> p t o", p=P))
            gmat_s = singles.tile([P, CT, G], F32)
            hmat_s = singles.tile([P, CT, P], F32)
            ident_s = singles.tile([P, P], BF16)
            nc.sync.dma_start(out=gmat_s, in_=gmat)
            nc.sync.dma_start(out=hmat_s, in_=hmat)
            nc.sync.dma_start(out=ident_s, in_=ident)
            gamma_s = singles.tile([P, CT], F32)
            beta_s = singles.tile([P, CT], F32)
            bq_s = singles.tile([P, CT], F32)
            bk_s = singles.tile([P, CT], F32)
            bo_s = singles.tile([P, CT], F32)
            nc.sync.dma_start(out=gamma_s, in_=pc(gamma))
            nc.sync.dma_start(out=beta_s, in_=pc(beta))
            nc.sync.dma_start(out=bq_s, in_=pc(bq))
            nc.sync.dma_start(out=bk_s, in_=pc(bk))
            nc.sync.dma_start(out=bo_s, in_=pc(bo))
            bv_b = singles.tile([P, C], F32)
            nc.sync.dma_start(
                out=bv_b,
                in_=bass.AP(tensor=bv.tensor, offset=bv.offset, ap=[[0, P], *bv.ap]),
            )

            # ---- PE warm-up: ~12us of tiny matmuls so the HAM clock
            # gate opens while batch 0's DMA + stats chain runs ----
            warm = singles.tile([P, 16], BF16)
            nc.vector.memset(warm, 1.0)
            pwarm = pmm.tile([P, 512], F32, tag="mm")
            for _ in range(430):
                nc.tensor.matmul(pwarm[:16, :16], warm, warm, start=True, stop=True)

            for b in range(BL):
                # ---- load x[b] as [p, t, n] (batch 0 already issued) ----
                if b == 0:
                    xs = xs_tiles[0]
                else:
                    xs = xpool.tile([P, CT, HW], F32, tag="xs")
                    xr = x[b].rearrange("(t p) n -> p t n", p=P)
                    for t in range(CT):
                        nc.sync.dma_start(out=xs[:, t, :], in_=xr[:, t, :])

                # ---- GroupNorm statistics ----
                # per-channel mean / E[x^2] via bn_stats over the free axis
                stat2 = small.tile([P, CT, 2], F32)
                for t in range(CT):
                    bnout = small.tile([P, 2, 6], F32)
                    xv = xs[:, t, :].rearrange("p (s f) -> p s f", f=512)
                    for s in range(2):
                        nc.vector.bn_stats(out=bnout[:, s, :], in_=xv[:, s, :])
                    nc.vector.bn_aggr(out=stat2[:, t, :], in_=bnout)
                # stat2[:,:,1] (var) += mean^2  ->  E[x^2]; then scale to sums
                sq = small.tile([P, CT], F32)
                nc.vector.tensor_mul(sq, stat2[:, :, 0], stat2[:, :, 0])
                nc.vector.tensor_add(stat2[:, :, 1], stat2[:, :, 1], sq)
                nc.vector.tensor_scalar_mul(stat2, stat2, float(HW))

                # group stats [32, 2] = sum_t gmat[:,t,:].T @ stat2[:,t,:]
                pp = pst.tile([P, 2 + CT * 2], F32)
                pg = pp[:G, 0:2]
                for t in range(CT):
                    nc.tensor.matmul(
                        pg,
                        gmat_s[:, t, :],
                        stat2[:, t, :],
                        start=(t == 0),
                        stop=(t == CT - 1),
                    )
                # rstd_g = 1/sqrt(E[x^2]-mean^2+eps);  mrs_g = mean*rstd
                gb = small.tile([P, 2], F32)
                nc.vector.memset(gb, 0.0)
                pgs = small.tile([G, 2], F32)
                nc.vector.tensor_copy(pgs, pg)
                msq = small.tile([G, 1], F32)
                nc.vector.tensor_mul(msq, pgs[:, 0:1], pgs[:, 0:1])
                veps = small.tile([G, 1], F32)
                nc.vector.tensor_scalar(
                    veps, pgs[:, 1:2], msq, EPS, op0=ALU.subtract, op1=ALU.add
                )
                std = small.tile([G, 1], F32)
                nc.scalar.activation(out=std, in_=veps, func=AF.Sqrt)
                nc.vector.reciprocal(gb[:G, 0:1], std)
                nc.vector.tensor_mul(gb[:G, 1:2], pgs[:, 0:1], gb[:G, 0:1])

                # broadcast group -> channel: [p, t, (rstd, mrs)]
                ppc = pp[:, 2:].rearrange("p (t k) -> p t k", k=2)
                for t in range(CT):
                    nc.tensor.matmul(
                        ppc[:, t, :], hmat_s[:, t, :], gb, start=True, stop=True
                    )
                # A = gamma * rstd ; Bb = beta - gamma * mean * rstd
                A = small.tile([P, CT], F32)
                Bb = small.tile([P, CT], F32)
                nc.vector.tensor_mul(A, gamma_s, ppc[:, :, 0])
                nc.vector.tensor_mul(Bb, gamma_s, ppc[:, :, 1])
                nc.vector.tensor_tensor(Bb, beta_s, Bb, op=ALU.subtract)

                # xb <- bf16(xs * A + Bb); xs stays raw, xn is recomputed
                # in fp32 at the residual step
                xb = acts.tile([P, CT, HW], BF16)
                for t in range(CT):
                    nc.vector.tensor_scalar(
                        xb[:, t, :],
                        xs[:, t, :],
                        A[:, t : t + 1],
                        Bb[:, t : t + 1],
                        op0=ALU.mult,
                        op1=ALU.add,
                    )

                # ---- q, k projections: [o, n] = W @ xn ----
                q_s = acts.tile([P, CT, HW], BF16)
                k_s = acts.tile([P, CT, HW], BF16)
                for (w_s, b_s, dst) in ((wq_s, bq_s, q_s), (wk_s, bk_s, k_s)):
                    for m in range(CT):
                        for nh in range(NHALF):
                            pqk = pmm.tile([P, 512], F32, tag="mm")
                            for t in range(CT):
                                nc.tensor.matmul(
                                    pqk,
                                    w_s[:, t, m * P : (m + 1) * P],
                                    xb[:, t, nh * 512 : (nh + 1) * 512],
                                    start=(t == 0),
                                    stop=(t == CT - 1),
                                )
                            nc.scalar.activation(
                                out=dst[:, m, nh * 512 : (nh + 1) * 512],
                                in_=pqk,
                                func=AF.Identity,
                                bias=b_s[:, m : m + 1],
                            )

                # ---- vT: [m, c] = xn^T @ WvT ----
                vT_s = acts.tile([P, NT, C], BF16)
                for j in range(NT):
                    pv = pmm.tile([P, 512], F32, tag="mm")
                    for t in range(CT):
                        nc.tensor.matmul(
                            pv,
                            xb[:, t, j * P : (j + 1) * P],
                            wv_s[:, t, :],
                            start=(t == 0),
                            stop=(t == CT - 1),
                        )
                    nc.vector.tensor_add(vT_s[:, j, :], pv, bv_b)

                # ---- attention ----
                # phase 1: scores + exp + row-normalize for all 8 n-blocks
                o2T = acts.tile([P, CT, HW], BF16)
                all_erows = []
                for i in range(NT):
                    asum = small.tile([P, 2], F32)
                    erow = erows.tile([P, HW], BF16)
                    for mh in range(NHALF):
                        ps = pmm.tile([P, 512], F32, tag="mm")
                        for t in range(CT):
                            nc.tensor.matmul(
                                ps,
                                q_s[:, t, i * P : (i + 1) * P],
                                k_s[:, t, mh * 512 : (mh + 1) * 512],
                                start=(t == 0),
                                stop=(t == CT - 1),
                            )
                        # exp(score/sqrt(C)); accumulate row sums in fp32
                        nc.scalar.activation(
                            out=erow[:, mh * 512 : (mh + 1) * 512],
                            in_=ps,
                            func=AF.Exp,
                            scale=SCALE,
                            accum_out=asum[:, mh : mh + 1],
                        )
                    den = small.tile([P, 1], F32)
                    nc.vector.tensor_add(den, asum[:, 0:1], asum[:, 1:2])
                    rec = small.tile([P, 1], F32)
                    nc.vector.reciprocal(rec, den)
                    nc.vector.tensor_scalar_mul(erow, erow, rec)
                    all_erows.append(erow)

                # phase 2 (per half): transpose, attn@v, output
                # projection + residual + per-half output store
                y_s = ypool.tile([P, CT, HW], F32)
                for nh in range(NHALF):
                    attnT = attnp.tile([P, NT, 512], BF16)
                    for ii in range(4):
                        erow = all_erows[nh * 4 + ii]
                        ptb = pmm.tile([P, NT, P], BF16, tag="mm")
                        for j in range(NT):
                            nc.tensor.transpose(
                                ptb[:, j, :],
                                erow[:, j * P : (j + 1) * P],
                                ident_s,
                            )
                        dst = attnT[:, :, ii * P : (ii + 1) * P]
                        if ii % 2 == 0:
                            nc.vector.tensor_copy(dst, ptb)
                        else:
                            nc.scalar.copy(dst, ptb)

                    # out2^T[c, n-half] = vT^T @ attnT
                    for cm in range(CT):
                        po = pmm.tile([P, 512], F32, tag="mm")
                        for j in range(NT):
                            nc.tensor.matmul(
                                po,
                                vT_s[:, j, cm * P : (cm + 1) * P],
                                attnT[:, j, :],
                                start=(j == 0),
                                stop=(j == NT - 1),
                            )
                        nc.vector.tensor_copy(
                            o2T[:, cm, nh * 512 : (nh + 1) * 512], po
                        )

                    # output projection + residual for this half
                    sl = slice(nh * 512, (nh + 1) * 512)
                    for m in range(CT):
                        pf = pmm.tile([P, 512], F32, tag="mm")
                        for t in range(CT):
                            nc.tensor.matmul(
                                pf,
                                wo_s[:, t, m * P : (m + 1) * P],
                                o2T[:, t, sl],
                                start=(t == 0),
                                stop=(t == CT - 1),
                            )
                        nc.scalar.activation(
                            out=pf,
                            in_=pf,
                            func=AF.Identity,
                            bias=bo_s[:, m : m + 1],
                        )
                        xnn = small.tile([P, 512], F32, tag="xnn")
                        nc.gpsimd.tensor_scalar(
                            xnn,
                            xs[:, m, sl],
                            A[:, m : m + 1],
                            Bb[:, m : m + 1],
                            op0=ALU.mult,
                            op1=ALU.add,
                        )
                        nc.vector.tensor_add(y_s[:, m, sl], pf, xnn)
                    # store this half as soon as it is done
                    nc.sync.dma_start(
                        out=y[b].rearrange("(t p) n -> p t n", p=P)[:, :, sl],
                        in_=y_s[:, :, sl],
                    )

    nc.compile()
    return nc


_NC_CACHE = None


def _get_module():
    global _NC_CACHE
    if _NC_CACHE is None:
        _NC_CACHE = build_module()
    return _NC_CACHE


def make_in_maps(x, gamma, beta, wq, bq, wk, bk, wv, bv, wo, bo):
    x = np.ascontiguousarray(np.asarray(x, dtype=np.float32)).reshape(B, C, HW)
    gmat, hmat, ident = _host_constants()

    def wt(w):  # transpose + bf16 for the stationary weight operand
        return np.ascontiguousarray(
            np.asarray(w, np.float32).T.astype(ml_dtypes.bfloat16)
        )

    shared = {
        "wqT": wt(wq),
        "wkT": wt(wk),
        "wvT": wt(wv),
        "woT": wt(wo),
        "gamma": np.asarray(gamma, np.float32),
        "beta": np.asarray(beta, np.float32),
        "bq": np.asarray(bq, np.float32),
        "bk": np.asarray(bk, np.float32),
        "bv": np.asarray(bv, np.float32),
        "bo": np.asarray(bo, np.float32),
        "gmat": gmat,
        "hmat": hmat,
        "ident": ident,
    }
    return [
        {"x": np.ascontiguousarray(x[c * BL : (c + 1) * BL]), **shared}
        for c in range(NCORES)
    ]


def run(inputs, trace=False, **kw):
    nc = _get_module()
    in_maps = make_in_maps(**inputs)
    res = run_bass_kernel_spmd(nc, in_maps, list(range(NCORES)), trace=trace, **kw)
    out = np.concatenate([res.results[c]["y"] for c in range(NCORES)], axis=0)
    return out.reshape(B, C, HH, WW), res


def kernel(**inputs):
    out, _ = run(inputs, trace=False)
    return out



# revision 6
# speedup vs baseline: 1.5589x; 1.5589x over previous
"""Trainium2 Bass kernel for an AttentionBlock (GroupNorm + single-head
self-attention over spatial positions + residual).

Reference computation (B=32, C=512, H=W=32, N=H*W=1024):
    xn = GroupNorm(32 groups)(x) * gamma + beta
    q/k/v = W{q,k,v} @ xn + b         (per batch, [C, N])
    score = q^T k / sqrt(C)           ([N, N])
    attn  = softmax(score, axis=-1)
    out   = Wo @ (v @ attn^T) + bo    ([C, N])
    y     = out + xn

Sharding: data-parallel over batch across 8 NeuronCores (4 batches each);
weights replicated.

Implementation notes:
- Softmax normalization is deferred to the very end (y = pf * recb + ...),
  which lets the whole attention block collapse algebraically to 4 GEMMs:
    t   = (Wk^T Wq) xn          scoresT = xn^T t   (+ per-key bias term)
    vt  = xn^T (Wo Wv)^T        pf      = vt^T erowT
  The composite weights M2 = Wk^T Wq and Wov = Wo Wv are formed on the
  host. The q-side bias terms are constant along the softmax axis and
  cancel; the k-side term u = Wk^T bq folds into the t copy-out; the
  v/o biases fold into bo2 = bo + Wo bv added with the residual.
- All GEMMs run in fp8 e4m3 with DoubleRow perf mode (2x throughput,
  256-deep contraction per instruction). Weights are pre-scaled x32 on
  the host so they quantize in e4m3's normal range; all scales are
  folded into copy-out constants and the deferred softmax reciprocal.
- GroupNorm statistics, softmax accumulation and the residual stay fp32.
  Measured end-to-end rel l2 error ~6e-3 (gate 2e-2).
"""

import os
import sys

for _p in ("/opt/trn_rl_repo", "/root/.axon_site/_ro/trn_rl_repo"):
    if os.path.isdir(_p) and _p not in sys.path:
        sys.path.insert(0, _p)

import numpy as np
import ml_dtypes

import concourse.bass as bass
import concourse.mybir as mybir
import concourse.tile as tile
from concourse import bacc
from concourse.bass_utils import run_bass_kernel_spmd

# Problem constants (hardcoded per harness contract)
B, C, HH, WW = 32, 512, 32, 32
HW = HH * WW                  # 1024 sequence positions
NCORES = 8
BL = B // NCORES              # batches per core
G = 32                        # groups
GS = C // G                   # channels per group (16)
P = 128                       # partitions
CT = C // P                   # channel chunks (4)
NT = HW // P                  # sequence chunks (8)
NHALF = HW // 512             # 512-wide free-dim halves (2)
EPS = 1e-5
SCALE = float(C) ** -0.5
WS = 32.0                     # fp8 weight pre-scale
C0 = 3.0                      # exp offset: erow = exp(score - C0)
SV = 0.5                      # vt copy-out scale (keeps 32*vt under e4m3 max 240)
ONESV = WS * SV               # den matmul constant; recb = 1/(ONESV*den)
F32 = mybir.dt.float32
BF16 = mybir.dt.bfloat16
FP8 = mybir.dt.float8e4
AF = mybir.ActivationFunctionType
ALU = mybir.AluOpType
DR = mybir.MatmulPerfMode.DoubleRow


def _host_constants():
    # gmat[p, t, g] = 1/(16*HW) if channel (t*128+p) is in group g
    gmat = np.zeros((P, CT, G), dtype=np.float32)
    # hmat[g, t, p] = 1 if channel (t*128+p) is in group g (group -> channel)
    hmat = np.zeros((P, CT, P), dtype=np.float32)
    for t in range(CT):
        for p in range(P):
            g = (t * P + p) // GS
            gmat[p, t, g] = 1.0 / (GS * HW)
            hmat[g, t, p] = 1.0
    return gmat, hmat


def build_module():
    nc = bacc.Bacc("TRN2", target_bir_lowering=False, debug=False)

    x = nc.dram_tensor("x", [BL, C, HW], F32, kind="ExternalInput").ap()
    y = nc.dram_tensor("y", [BL, C, HW], F32, kind="ExternalOutput").ap()
    m2T = nc.dram_tensor("m2T", [C, C], FP8, kind="ExternalInput").ap()
    wovT = nc.dram_tensor("wovT", [C, C], FP8, kind="ExternalInput").ap()
    gamma = nc.dram_tensor("gamma", [C], F32, kind="ExternalInput").ap()
    beta = nc.dram_tensor("beta", [C], F32, kind="ExternalInput").ap()
    uvec = nc.dram_tensor("uvec", [C], F32, kind="ExternalInput").ap()
    bo2 = nc.dram_tensor("bo2", [C], F32, kind="ExternalInput").ap()
    gmat = nc.dram_tensor("gmat", [P, CT, G], F32, kind="ExternalInput").ap()
    hmat = nc.dram_tensor("hmat", [P, CT, P], F32, kind="ExternalInput").ap()

    def pc(v):  # [C] dram -> [P, CT] sbuf layout (channel c = t*128+p)
        return v.rearrange("(t p) -> p t", p=P)

    with tile.TileContext(nc) as tc:
        with (
            tc.tile_pool(name="singles", bufs=1) as singles,
            tc.tile_pool(name="xpool", bufs=2) as xpool,
            tc.tile_pool(name="acts", bufs=2) as acts,
            tc.tile_pool(name="ypool", bufs=2) as ypool,
            tc.tile_pool(name="small", bufs=4) as small,
            tc.tile_pool(name="pbig", bufs=3, space="PSUM") as pbig,
            tc.tile_pool(name="pden", bufs=1, space="PSUM") as pden,
        ):
            # ---- batch-0 input first: its stats chain is the critical path ----
            xs_tiles = []
            xs0 = xpool.tile([P, CT, HW], F32, tag="xs")
            xs_tiles.append(xs0)
            x0r = x[0].rearrange("(t p) n -> p t n", p=P)
            for t in range(CT):
                nc.sync.dma_start(out=xs0[:, t, :], in_=x0r[:, t, :])

            # ---- load constants / weights once ----
            m2_s = singles.tile([P, CT, C], FP8)
            wov_s = singles.tile([P, CT, C], FP8)
            nc.sync.dma_start(out=m2_s, in_=m2T.rearrange("(t p) o -> p t o", p=P))
            nc.sync.dma_start(out=wov_s, in_=wovT.rearrange("(t p) o -> p t o", p=P))
            gmat_s = singles.tile([P, CT, G], F32)
            hmat_s = singles.tile([P, CT, P], F32)
            nc.sync.dma_start(out=gmat_s, in_=gmat)
            nc.sync.dma_start(out=hmat_s, in_=hmat)
            gamma_s = singles.tile([P, CT], F32)
            beta_s = singles.tile([P, CT], F32)
            u_s = singles.tile([P, CT], F32)
            bo2_s = singles.tile([P, CT], F32)
            nc.sync.dma_start(out=gamma_s, in_=pc(gamma))
            nc.sync.dma_start(out=beta_s, in_=pc(beta))
            nc.sync.dma_start(out=u_s, in_=pc(uvec))
            nc.sync.dma_start(out=bo2_s, in_=pc(bo2))
            ones_s = singles.tile([P, 2, P], FP8)
            nc.vector.memset(ones_s, ONESV)
            negc0 = singles.tile([P, 1], F32)
            nc.vector.memset(negc0, -C0)

            # ---- PE warm-up: tiny matmuls so the HAM clock gate opens
            # while batch 0's DMA + stats chain runs ----
            warm = singles.tile([P, 16], BF16)
            nc.vector.memset(warm, 1.0)
            pwarm = pbig.tile([P, 1024], F32, tag="mm")
            for _ in range(430):
                nc.tensor.matmul(pwarm[:16, :16], warm, warm, start=True, stop=True)

            for b in range(BL):
                # ---- load x[b] as [p, t, n] (batch 0 already issued) ----
                if b == 0:
                    xs = xs_tiles[0]
                else:
                    xs = xpool.tile([P, CT, HW], F32, tag="xs")
                    xr = x[b].rearrange("(t p) n -> p t n", p=P)
                    for t in range(CT):
                        nc.sync.dma_start(out=xs[:, t, :], in_=xr[:, t, :])

                # ---- GroupNorm statistics ----
                # per-channel mean / E[x^2] via bn_stats over the free axis
                stat2 = small.tile([P, CT, 2], F32, tag="stat2")
                for t in range(CT):
                    bnout = small.tile([P, 2, 6], F32, tag="bnout")
                    xv = xs[:, t, :].rearrange("p (s f) -> p s f", f=512)
                    for s in range(2):
                        nc.vector.bn_stats(out=bnout[:, s, :], in_=xv[:, s, :])
                    nc.vector.bn_aggr(out=stat2[:, t, :], in_=bnout)
                # stat2[:,:,1] (var) += mean^2  ->  E[x^2]; then scale to sums
                sq = small.tile([P, CT], F32, tag="sq")
                nc.vector.tensor_mul(sq, stat2[:, :, 0], stat2[:, :, 0])
                nc.vector.tensor_add(stat2[:, :, 1], stat2[:, :, 1], sq)
                nc.vector.tensor_scalar_mul(stat2, stat2, float(HW))

                # group stats [32, 2] = sum_t gmat[:,t,:].T @ stat2[:,t,:]
                pp = pbig.tile([P, 1024], F32, tag="mm")
                pg = pp[:G, 0:2]
                for t in range(CT):
                    nc.tensor.matmul(
                        pg,
                        gmat_s[:, t, :],
                        stat2[:, t, :],
                        start=(t == 0),
                        stop=(t == CT - 1),
                    )
                # rstd_g = 1/sqrt(E[x^2]-mean^2+eps);  mrs_g = mean*rstd
                gb = small.tile([P, 2], F32, tag="gb")
                nc.vector.memset(gb, 0.0)
                pgs = small.tile([G, 2], F32, tag="pgs")
                nc.vector.tensor_copy(pgs, pg)
                msq = small.tile([G, 1], F32, tag="msq")
                nc.vector.tensor_mul(msq, pgs[:, 0:1], pgs[:, 0:1])
                veps = small.tile([G, 1], F32, tag="veps")
                nc.vector.tensor_scalar(
                    veps, pgs[:, 1:2], msq, EPS, op0=ALU.subtract, op1=ALU.add
                )
                std = small.tile([G, 1], F32, tag="std")
                nc.scalar.activation(out=std, in_=veps, func=AF.Sqrt)
                nc.vector.reciprocal(gb[:G, 0:1], std)
                nc.vector.tensor_mul(gb[:G, 1:2], pgs[:, 0:1], gb[:G, 0:1])

                # broadcast group -> channel: [p, t, (rstd, mrs)]
                ppc = pp[:, 512 : 512 + 2 * CT].rearrange("p (t k) -> p t k", k=2)
                for t in range(CT):
                    nc.tensor.matmul(
                        ppc[:, t, :], hmat_s[:, t, :], gb, start=True, stop=True
                    )
                # A = gamma * rstd ; Bb = beta - gamma * mean * rstd
                A = acts.tile([P, CT], F32, tag="A")
                Bb = acts.tile([P, CT], F32, tag="Bb")
                Bb2 = acts.tile([P, CT], F32, tag="Bb2")
                nc.vector.tensor_mul(A, gamma_s, ppc[:, :, 0])
                nc.vector.tensor_mul(Bb, gamma_s, ppc[:, :, 1])
                nc.vector.tensor_tensor(Bb, beta_s, Bb, op=ALU.subtract)
                nc.vector.tensor_add(Bb2, Bb, bo2_s)

                # xb8 <- fp8(xs * A + Bb) on scalar; xbo <- f32 xn + bo2 on
                # gpsimd (residual + folded v/o bias)
                xb8 = acts.tile([P, CT, HW], FP8, tag="xb8")
                xbo = acts.tile([P, CT, HW], F32, tag="xbo")
                for t in range(CT):
                    nc.scalar.activation(
                        out=xb8[:, t, :],
                        in_=xs[:, t, :],
                        func=AF.Identity,
                        scale=A[:, t : t + 1],
                        bias=Bb[:, t : t + 1],
                    )
                    nc.gpsimd.tensor_scalar(
                        xbo[:, t, :],
                        xs[:, t, :],
                        A[:, t : t + 1],
                        Bb2[:, t : t + 1],
                        op0=ALU.mult,
                        op1=ALU.add,
                    )

                # ---- t = M2 @ xn  (+u fold): t8[c, n] fp8 ----
                t8 = acts.tile([P, CT, HW], FP8, tag="t8")
                for ob in range(CT):
                    pt = pbig.tile([P, 1024], F32, tag="mm")
                    for nh in range(NHALF):
                        for tt in (0, 2):
                            nc.tensor.matmul(
                                pt[:, nh * 512 : (nh + 1) * 512],
                                m2_s[:, tt : tt + 2, ob * P : (ob + 1) * P],
                                xb8[:, tt : tt + 2, nh * 512 : (nh + 1) * 512],
                                start=(tt == 0),
                                stop=(tt == 2),
                                perf_mode=DR,
                            )
                    # t8 = psum/32 + u  (u = Wk^T bq; zero in the common case)
                    nc.vector.tensor_scalar(
                        t8[:, ob, :],
                        pt,
                        1.0 / WS,
                        u_s[:, ob : ob + 1],
                        op0=ALU.mult,
                        op1=ALU.add,
                    )

                # ---- scoresT + exp (per m-block), interleaved with
                # vt = xn^T Wov^T chains (independent of t8) ----
                erow = acts.tile([P, NT, HW], FP8, tag="erow")
                vt8 = acts.tile([P, NT, C], FP8, tag="vt8")

                def sc_block(i):
                    ps = pbig.tile([P, 1024], F32, tag="mm", name=f"ps{b}_{i}")
                    for nh in range(NHALF):
                        for tt in (0, 2):
                            nc.tensor.matmul(
                                ps[:, nh * 512 : (nh + 1) * 512],
                                xb8[:, tt : tt + 2, i * P : (i + 1) * P],
                                t8[:, tt : tt + 2, nh * 512 : (nh + 1) * 512],
                                start=(tt == 0),
                                stop=(tt == 2),
                                perf_mode=DR,
                            )
                    nc.scalar.activation(
                        out=erow[:, i, :],
                        in_=ps,
                        func=AF.Exp,
                        scale=SCALE,
                        bias=negc0,
                    )

                def vt_pair(j):
                    pv = pbig.tile([P, 1024], F32, tag="mm", name=f"pv{b}_{j}")
                    for jj in (j, j + 1):
                        for tt in (0, 2):
                            nc.tensor.matmul(
                                pv[:, (jj - j) * 512 : (jj - j + 1) * 512],
                                xb8[:, tt : tt + 2, jj * P : (jj + 1) * P],
                                wov_s[:, tt : tt + 2, :],
                                start=(tt == 0),
                                stop=(tt == 2),
                                perf_mode=DR,
                            )
                    # vt8 = SV * psum  (carries WS*SV = 16x true vt)
                    nc.vector.tensor_scalar_mul(vt8[:, j : j + 2, :], pv, SV)

                sc_block(0)
                vt_pair(0)
                sc_block(1)
                sc_block(2)
                vt_pair(2)
                sc_block(3)
                sc_block(4)
                vt_pair(4)
                sc_block(5)
                sc_block(6)
                vt_pair(6)
                sc_block(7)

                # ---- den (deferred softmax denominator), broadcast over
                # partitions by an all-16s stationary ----
                pd = pden.tile([P, 1024], F32, tag="pd")
                for nh in range(NHALF):
                    for jj in (0, 2, 4, 6):
                        nc.tensor.matmul(
                            pd[:, nh * 512 : (nh + 1) * 512],
                            ones_s,
                            erow[:, jj : jj + 2, nh * 512 : (nh + 1) * 512],
                            start=(jj == 0),
                            stop=(jj == 6),
                            perf_mode=DR,
                        )
                recb = acts.tile([P, HW], F32, tag="recb")
                nc.vector.reciprocal_approx_fast(out=recb, in_=pd)

                # ---- attention output + residual ----
                y_s = ypool.tile([P, CT, HW], F32, tag="ys")
                yr = y[b].rearrange("(t p) n -> p t n", p=P)
                for ob in range(CT):
                    pf = pbig.tile([P, 1024], F32, tag="mm", name=f"pf{b}_{ob}")
                    for nh in range(NHALF):
                        for jj in (0, 2, 4, 6):
                            nc.tensor.matmul(
                                pf[:, nh * 512 : (nh + 1) * 512],
                                vt8[:, jj : jj + 2, ob * P : (ob + 1) * P],
                                erow[:, jj : jj + 2, nh * 512 : (nh + 1) * 512],
                                start=(jj == 0),
                                stop=(jj == 6),
                                perf_mode=DR,
                            )
                    nc.vector.tensor_tensor(y_s[:, ob, :], pf, recb, op=ALU.mult)
                    nc.gpsimd.tensor_tensor(
                        y_s[:, ob, :], y_s[:, ob, :], xbo[:, ob, :], op=ALU.add
                    )
                    nc.sync.dma_start(out=yr[:, ob, :], in_=y_s[:, ob, :])

    nc.compile()
    return nc


_NC_CACHE = None


def _get_module():
    global _NC_CACHE
    if _NC_CACHE is None:
        _NC_CACHE = build_module()
    return _NC_CACHE


def make_in_maps(x, gamma, beta, wq, bq, wk, bk, wv, bv, wo, bo):
    x = np.ascontiguousarray(np.asarray(x, dtype=np.float32)).reshape(B, C, HW)
    gmat, hmat = _host_constants()

    f64 = lambda a: np.asarray(a, np.float64)
    wq64, wk64, wv64, wo64 = f64(wq), f64(wk), f64(wv), f64(wo)
    # composite weights (see module docstring); pre-scaled x32 for e4m3
    m2T = np.ascontiguousarray(
        ((wq64.T @ wk64) * WS).astype(np.float32).astype(ml_dtypes.float8_e4m3)
    )
    wovT = np.ascontiguousarray(
        (((wo64 @ wv64).T) * WS).astype(np.float32).astype(ml_dtypes.float8_e4m3)
    )
    uvec = (wk64.T @ f64(bq)).astype(np.float32)
    bo2 = (f64(bo) + wo64 @ f64(bv)).astype(np.float32)

    shared = {
        "m2T": m2T,
        "wovT": wovT,
        "gamma": np.asarray(gamma, np.float32),
        "beta": np.asarray(beta, np.float32),
        "uvec": uvec,
        "bo2": bo2,
        "gmat": gmat,
        "hmat": hmat,
    }
    return [
        {"x": np.ascontiguousarray(x[c * BL : (c + 1) * BL]), **shared}
        for c in range(NCORES)
    ]


def run(inputs, trace=False, **kw):
    nc = _get_module()
    in_maps = make_in_maps(**inputs)
    res = run_bass_kernel_spmd(nc, in_maps, list(range(NCORES)), trace=trace, **kw)
    out = np.concatenate([res.results[c]["y"] for c in range(NCORES)], axis=0)
    return out.reshape(B, C, HH, WW), res


def kernel(**inputs):
    out, _ = run(inputs, trace=False)
    return out


# revision 10
# speedup vs baseline: 1.5605x; 1.0010x over previous
"""Trainium2 Bass kernel for an AttentionBlock (GroupNorm + single-head
self-attention over spatial positions + residual).

Reference computation (B=32, C=512, H=W=32, N=H*W=1024):
    xn = GroupNorm(32 groups)(x) * gamma + beta
    q/k/v = W{q,k,v} @ xn + b         (per batch, [C, N])
    score = q^T k / sqrt(C)           ([N, N])
    attn  = softmax(score, axis=-1)
    out   = Wo @ (v @ attn^T) + bo    ([C, N])
    y     = out + xn

Sharding: data-parallel over batch across 8 NeuronCores (4 batches each);
weights replicated.

Implementation notes:
- Softmax normalization is deferred to the very end (y = pf * recb + ...),
  which lets the whole attention block collapse algebraically to 4 GEMMs:
    t   = (Wk^T Wq) xn          scoresT = xn^T t   (+ per-key bias term)
    vt  = xn^T (Wo Wv)^T        pf      = vt^T erowT
  The composite weights M2 = Wk^T Wq and Wov = Wo Wv are formed on the
  host. The q-side bias terms are constant along the softmax axis and
  cancel; the k-side term u = Wk^T bq folds into the t copy-out; the
  v/o biases fold into bo2 = bo + Wo bv added with the residual.
- All GEMMs run in fp8 e4m3 with DoubleRow perf mode (2x throughput,
  256-deep contraction per instruction). Weights are pre-scaled x32 on
  the host so they quantize in e4m3's normal range; all scales are
  folded into copy-out constants and the deferred softmax reciprocal.
- GroupNorm statistics, softmax accumulation and the residual stay fp32.
  Measured end-to-end rel l2 error ~6e-3 (gate 2e-2).
"""

import os
import sys

for _p in ("/opt/trn_rl_repo", "/root/.axon_site/_ro/trn_rl_repo"):
    if os.path.isdir(_p) and _p not in sys.path:
        sys.path.insert(0, _p)

import numpy as np
import ml_dtypes

import concourse.bass as bass
import concourse.mybir as mybir
import concourse.tile as tile
from concourse import bacc
from concourse.bass_utils import run_bass_kernel_spmd

# Problem constants (hardcoded per harness contract)
B, C, HH, WW = 32, 512, 32, 32
HW = HH * WW                  # 1024 sequence positions
NCORES = 8
BL = B // NCORES              # batches per core
G = 32                        # groups
GS = C // G                   # channels per group (16)
P = 128                       # partitions
CT = C // P                   # channel chunks (4)
NT = HW // P                  # sequence chunks (8)
NHALF = HW // 512             # 512-wide free-dim halves (2)
EPS = 1e-5
SCALE = float(C) ** -0.5
WS = 32.0                     # fp8 weight pre-scale
C0 = 3.0                      # exp offset: erow = exp(score - C0)
SV = 0.5                      # vt copy-out scale (keeps 32*vt under e4m3 max 240)
ONESV = WS * SV               # den matmul constant; recb = 1/(ONESV*den)
F32 = mybir.dt.float32
BF16 = mybir.dt.bfloat16
FP8 = mybir.dt.float8e4
AF = mybir.ActivationFunctionType
ALU = mybir.AluOpType
DR = mybir.MatmulPerfMode.DoubleRow


def _host_constants():
    # gmat[p, t, g] = 1/(16*HW) if channel (t*128+p) is in group g
    gmat = np.zeros((P, CT, G), dtype=np.float32)
    # hmat[g, t, p] = 1 if channel (t*128+p) is in group g (group -> channel)
    hmat = np.zeros((P, CT, P), dtype=np.float32)
    for t in range(CT):
        for p in range(P):
            g = (t * P + p) // GS
            gmat[p, t, g] = 1.0 / (GS * HW)
            hmat[g, t, p] = 1.0
    return gmat, hmat


def build_module():
    nc = bacc.Bacc("TRN2", target_bir_lowering=False, debug=False)

    x = nc.dram_tensor("x", [BL, C, HW], F32, kind="ExternalInput").ap()
    y = nc.dram_tensor("y", [BL, C, HW], F32, kind="ExternalOutput").ap()
    m2T = nc.dram_tensor("m2T", [C, C], FP8, kind="ExternalInput").ap()
    wovT = nc.dram_tensor("wovT", [C, C], FP8, kind="ExternalInput").ap()
    gamma = nc.dram_tensor("gamma", [C], F32, kind="ExternalInput").ap()
    beta = nc.dram_tensor("beta", [C], F32, kind="ExternalInput").ap()
    uvec = nc.dram_tensor("uvec", [C], F32, kind="ExternalInput").ap()
    bo2 = nc.dram_tensor("bo2", [C], F32, kind="ExternalInput").ap()
    gmat = nc.dram_tensor("gmat", [P, CT, G], F32, kind="ExternalInput").ap()
    hmat = nc.dram_tensor("hmat", [P, CT, P], F32, kind="ExternalInput").ap()

    def pc(v):  # [C] dram -> [P, CT] sbuf layout (channel c = t*128+p)
        return v.rearrange("(t p) -> p t", p=P)

    with tile.TileContext(nc) as tc:
        with (
            tc.tile_pool(name="singles", bufs=1) as singles,
            tc.tile_pool(name="xpool", bufs=3) as xpool,
            tc.tile_pool(name="acts", bufs=2) as acts,
            tc.tile_pool(name="ypool", bufs=2) as ypool,
            tc.tile_pool(name="small", bufs=4) as small,
            tc.tile_pool(name="pbig", bufs=4, space="PSUM") as pbig,
        ):
            # ---- batch 0/1 inputs first: stats chains are the critical path ----
            xs_t = {}

            def emit_load(b):
                xs = xpool.tile([P, CT, HW], F32, tag="xs", name=f"xs{b}")
                xr = x[b].rearrange("(t p) n -> p t n", p=P)
                for t in range(CT):
                    nc.sync.dma_start(out=xs[:, t, :], in_=xr[:, t, :])
                xs_t[b] = xs

            emit_load(0)
            emit_load(1)

            # ---- load constants / weights once ----
            m2_s = singles.tile([P, CT, C], FP8)
            wov_s = singles.tile([P, CT, C], FP8)
            nc.sync.dma_start(out=m2_s, in_=m2T.rearrange("(t p) o -> p t o", p=P))
            nc.sync.dma_start(out=wov_s, in_=wovT.rearrange("(t p) o -> p t o", p=P))
            gmat_s = singles.tile([P, CT, G], F32)
            hmat_s = singles.tile([P, CT, P], F32)
            nc.sync.dma_start(out=gmat_s, in_=gmat)
            nc.sync.dma_start(out=hmat_s, in_=hmat)
            gamma_s = singles.tile([P, CT], F32)
            beta_s = singles.tile([P, CT], F32)
            u_s = singles.tile([P, CT], F32)
            bo2_s = singles.tile([P, CT], F32)
            nc.sync.dma_start(out=gamma_s, in_=pc(gamma))
            nc.sync.dma_start(out=beta_s, in_=pc(beta))
            nc.sync.dma_start(out=u_s, in_=pc(uvec))
            nc.sync.dma_start(out=bo2_s, in_=pc(bo2))
            ones_s = singles.tile([P, 2, P], FP8)
            nc.vector.memset(ones_s, ONESV)
            negc0 = singles.tile([P, 1], F32)
            nc.vector.memset(negc0, -C0)

            # ---- PE warm-up: tiny matmuls so the HAM clock gate opens
            # while batch 0's DMA + stats chain runs ----
            warm = singles.tile([P, 16], BF16)
            nc.vector.memset(warm, 1.0)
            pwarm = pbig.tile([P, 1024], F32, tag="mm")
            for _ in range(430):
                nc.tensor.matmul(pwarm[:16, :16], warm, warm, start=True, stop=True)

            st = {}   # per-batch state: xb8, xbo, t8, erow, vt8

            def emit_stats_xb(b):
                """GroupNorm stats -> A/Bb, then xb8 (vector) + xbo (gpsimd)."""
                xs = xs_t[b]
                stat2 = small.tile([P, CT, 2], F32, tag="stat2", name=f"st{b}")
                for t in range(CT):
                    bnout = small.tile([P, 2, 6], F32, tag="bnout", name=f"bn{b}_{t}")
                    xv = xs[:, t, :].rearrange("p (s f) -> p s f", f=512)
                    for s in range(2):
                        nc.vector.bn_stats(out=bnout[:, s, :], in_=xv[:, s, :])
                    nc.vector.bn_aggr(out=stat2[:, t, :], in_=bnout)
                sq = small.tile([P, CT], F32, tag="sq", name=f"sq{b}")
                nc.vector.tensor_mul(sq, stat2[:, :, 0], stat2[:, :, 0])
                nc.vector.tensor_add(stat2[:, :, 1], stat2[:, :, 1], sq)
                nc.vector.tensor_scalar_mul(stat2, stat2, float(HW))

                # group stats [32, 2] = sum_t gmat[:,t,:].T @ stat2[:,t,:]
                pp = pbig.tile([P, 1024], F32, tag="mm", name=f"pp{b}")
                pg = pp[:G, 0:2]
                for t in range(CT):
                    nc.tensor.matmul(
                        pg,
                        gmat_s[:, t, :],
                        stat2[:, t, :],
                        start=(t == 0),
                        stop=(t == CT - 1),
                    )
                # rstd_g = 1/sqrt(E[x^2]-mean^2+eps);  mrs_g = mean*rstd
                gb = small.tile([P, 2], F32, tag="gb", name=f"gb{b}")
                nc.vector.memset(gb, 0.0)
                pgs = small.tile([G, 2], F32, tag="pgs", name=f"pgs{b}")
                nc.vector.tensor_copy(pgs, pg)
                msq = small.tile([G, 1], F32, tag="msq", name=f"msq{b}")
                nc.vector.tensor_mul(msq, pgs[:, 0:1], pgs[:, 0:1])
                veps = small.tile([G, 1], F32, tag="veps", name=f"veps{b}")
                nc.vector.tensor_scalar(
                    veps, pgs[:, 1:2], msq, EPS, op0=ALU.subtract, op1=ALU.add
                )
                std = small.tile([G, 1], F32, tag="std", name=f"sd{b}")
                nc.scalar.activation(out=std, in_=veps, func=AF.Sqrt)
                nc.vector.reciprocal(gb[:G, 0:1], std)
                nc.vector.tensor_mul(gb[:G, 1:2], pgs[:, 0:1], gb[:G, 0:1])

                # broadcast group -> channel: [p, t, (rstd, mrs)]
                ppc = pp[:, 512 : 512 + 2 * CT].rearrange("p (t k) -> p t k", k=2)
                for t in range(CT):
                    nc.tensor.matmul(
                        ppc[:, t, :], hmat_s[:, t, :], gb, start=True, stop=True
                    )
                # A = gamma * rstd ; Bb = beta - gamma * mean * rstd
                A = acts.tile([P, CT], F32, tag="A", name=f"A{b}")
                Bb = acts.tile([P, CT], F32, tag="Bb", name=f"Bb{b}")
                Bb2 = acts.tile([P, CT], F32, tag="Bb2", name=f"Bb2{b}")
                nc.vector.tensor_mul(A, gamma_s, ppc[:, :, 0])
                nc.vector.tensor_mul(Bb, gamma_s, ppc[:, :, 1])
                nc.vector.tensor_tensor(Bb, beta_s, Bb, op=ALU.subtract)
                nc.vector.tensor_add(Bb2, Bb, bo2_s)

                # xb8 <- fp8(xs*A + Bb) on vector; xbo <- f32 xn + bo2 on
                # gpsimd (residual + folded v/o bias)
                xb8 = acts.tile([P, CT, HW], FP8, tag="xb8", name=f"xb8{b}")
                xbo = acts.tile([P, CT, HW], F32, tag="xbo", name=f"xbo{b}")
                for t in range(CT):
                    nc.vector.tensor_scalar(
                        xb8[:, t, :],
                        xs[:, t, :],
                        A[:, t : t + 1],
                        Bb[:, t : t + 1],
                        op0=ALU.mult,
                        op1=ALU.add,
                    )
                    nc.gpsimd.tensor_scalar(
                        xbo[:, t, :],
                        xs[:, t, :],
                        A[:, t : t + 1],
                        Bb2[:, t : t + 1],
                        op0=ALU.mult,
                        op1=ALU.add,
                    )
                st[b] = {"xb8": xb8, "xbo": xbo}

            def emit_t(b):
                """t = M2 @ xn (+u fold): t8[c, n] fp8. Fills the previous
                batch's exp tail on the tensor engine."""
                xb8 = st[b]["xb8"]
                t8 = acts.tile([P, CT, HW], FP8, tag="t8", name=f"t8{b}")
                for ob in range(CT):
                    pt = pbig.tile([P, 1024], F32, tag="mm", name=f"pt{b}_{ob}")
                    for nh in range(NHALF):
                        for tt in (0, 2):
                            nc.tensor.matmul(
                                pt[:, nh * 512 : (nh + 1) * 512],
                                m2_s[:, tt : tt + 2, ob * P : (ob + 1) * P],
                                xb8[:, tt : tt + 2, nh * 512 : (nh + 1) * 512],
                                start=(tt == 0),
                                stop=(tt == 2),
                                perf_mode=DR,
                            )
                    # t8 = psum/32 + u  (u = Wk^T bq; zero in the common case)
                    if ob % 2 == 0:
                        nc.vector.tensor_scalar(
                            t8[:, ob, :],
                            pt,
                            1.0 / WS,
                            u_s[:, ob : ob + 1],
                            op0=ALU.mult,
                            op1=ALU.add,
                        )
                    else:
                        nc.scalar.activation(
                            out=t8[:, ob, :],
                            in_=pt,
                            func=AF.Identity,
                            scale=1.0 / WS,
                            bias=u_s[:, ob : ob + 1],
                        )
                st[b]["t8"] = t8

            def emit_vt_sc(b):
                """vt chains (front, covers t8-copy latency), then scoresT
                chains + exp per m-block."""
                xb8 = st[b]["xb8"]
                t8 = st[b]["t8"]
                erow = acts.tile([P, NT, HW], FP8, tag="erow", name=f"er{b}")
                vt8 = acts.tile([P, NT, C], FP8, tag="vt8", name=f"vt{b}")

                for j in (0, 2, 4, 6):
                    pv = pbig.tile([P, 1024], F32, tag="mm", name=f"pv{b}_{j}")
                    for jj in (j, j + 1):
                        for tt in (0, 2):
                            nc.tensor.matmul(
                                pv[:, (jj - j) * 512 : (jj - j + 1) * 512],
                                xb8[:, tt : tt + 2, jj * P : (jj + 1) * P],
                                wov_s[:, tt : tt + 2, :],
                                start=(tt == 0),
                                stop=(tt == 2),
                                perf_mode=DR,
                            )
                    # vt8 = SV * psum  (carries WS*SV = 16x true vt)
                    if j % 4 == 0:
                        nc.scalar.activation(
                            out=vt8[:, j : j + 2, :], in_=pv, func=AF.Identity, scale=SV
                        )
                    else:
                        nc.vector.tensor_scalar_mul(vt8[:, j : j + 2, :], pv, SV)

                for i in range(NT):
                    ps = pbig.tile([P, 1024], F32, tag="mm", name=f"ps{b}_{i}")
                    for nh in range(NHALF):
                        for tt in (0, 2):
                            nc.tensor.matmul(
                                ps[:, nh * 512 : (nh + 1) * 512],
                                xb8[:, tt : tt + 2, i * P : (i + 1) * P],
                                t8[:, tt : tt + 2, nh * 512 : (nh + 1) * 512],
                                start=(tt == 0),
                                stop=(tt == 2),
                                perf_mode=DR,
                            )
                    nc.scalar.activation(
                        out=erow[:, i, :],
                        in_=ps,
                        func=AF.Exp,
                        scale=SCALE,
                        bias=negc0,
                    )
                st[b]["erow"] = erow
                st[b]["vt8"] = vt8

            def emit_den_attn(b):
                erow = st[b]["erow"]
                vt8 = st[b]["vt8"]
                xbo = st[b]["xbo"]
                # den (deferred softmax denominator), broadcast over
                # partitions by an all-16s stationary
                pd = pbig.tile([P, 1024], F32, tag="mm", name=f"pd{b}")
                for nh in range(NHALF):
                    for jj in (0, 2, 4, 6):
                        nc.tensor.matmul(
                            pd[:, nh * 512 : (nh + 1) * 512],
                            ones_s,
                            erow[:, jj : jj + 2, nh * 512 : (nh + 1) * 512],
                            start=(jj == 0),
                            stop=(jj == 6),
                            perf_mode=DR,
                        )
                recb = acts.tile([P, HW], F32, tag="recb", name=f"rb{b}")
                nc.vector.reciprocal_approx_fast(out=recb, in_=pd)

                # attention output + residual
                y_s = ypool.tile([P, CT, HW], F32, tag="ys", name=f"ys{b}")
                yr = y[b].rearrange("(t p) n -> p t n", p=P)
                for ob in range(CT):
                    pf = pbig.tile([P, 1024], F32, tag="mm", name=f"pf{b}_{ob}")
                    for nh in range(NHALF):
                        for jj in (0, 2, 4, 6):
                            nc.tensor.matmul(
                                pf[:, nh * 512 : (nh + 1) * 512],
                                vt8[:, jj : jj + 2, ob * P : (ob + 1) * P],
                                erow[:, jj : jj + 2, nh * 512 : (nh + 1) * 512],
                                start=(jj == 0),
                                stop=(jj == 6),
                                perf_mode=DR,
                            )
                    nc.vector.tensor_tensor(y_s[:, ob, :], pf, recb, op=ALU.mult)
                    nc.gpsimd.tensor_tensor(
                        y_s[:, ob, :], y_s[:, ob, :], xbo[:, ob, :], op=ALU.add
                    )
                    nc.sync.dma_start(out=yr[:, ob, :], in_=y_s[:, ob, :])
                del st[b]

            # ---- software-pipelined batch loop: batch b+1's stats/xb8/t
            # fill batch b's exp tail on the tensor engine, so the PE never
            # idles > the HAM re-throttle window ----
            emit_stats_xb(0)
            emit_t(0)
            for b in range(BL):
                emit_vt_sc(b)
                if b + 1 < BL:
                    if b + 2 < BL:
                        emit_load(b + 2)
                    emit_stats_xb(b + 1)
                    emit_t(b + 1)
                emit_den_attn(b)

    nc.compile()
    return nc


_NC_CACHE = None


def _get_module():
    global _NC_CACHE
    if _NC_CACHE is None:
        _NC_CACHE = build_module()
    return _NC_CACHE


def make_in_maps(x, gamma, beta, wq, bq, wk, bk, wv, bv, wo, bo):
    x = np.ascontiguousarray(np.asarray(x, dtype=np.float32)).reshape(B, C, HW)
    gmat, hmat = _host_constants()

    f64 = lambda a: np.asarray(a, np.float64)
    wq64, wk64, wv64, wo64 = f64(wq), f64(wk), f64(wv), f64(wo)
    # composite weights (see module docstring); pre-scaled x32 for e4m3
    m2T = np.ascontiguousarray(
        ((wq64.T @ wk64) * WS).astype(np.float32).astype(ml_dtypes.float8_e4m3)
    )
    wovT = np.ascontiguousarray(
        (((wo64 @ wv64).T) * WS).astype(np.float32).astype(ml_dtypes.float8_e4m3)
    )
    uvec = (wk64.T @ f64(bq)).astype(np.float32)
    bo2 = (f64(bo) + wo64 @ f64(bv)).astype(np.float32)

    shared = {
        "m2T": m2T,
        "wovT": wovT,
        "gamma": np.asarray(gamma, np.float32),
        "beta": np.asarray(beta, np.float32),
        "uvec": uvec,
        "bo2": bo2,
        "gmat": gmat,
        "hmat": hmat,
    }
    return [
        {"x": np.ascontiguousarray(x[c * BL : (c + 1) * BL]), **shared}
        for c in range(NCORES)
    ]


def run(inputs, trace=False, **kw):
    nc = _get_module()
    in_maps = make_in_maps(**inputs)
    res = run_bass_kernel_spmd(nc, in_maps, list(range(NCORES)), trace=trace, **kw)
    out = np.concatenate([res.results[c]["y"] for c in range(NCORES)], axis=0)
    return out.reshape(B, C, HH, WW), res


def kernel(**inputs):
    out, _ = run(inputs, trace=False)
    return out


# revision 12
# speedup vs baseline: 1.6766x; 1.0744x over previous
"""Trainium2 Bass kernel for an AttentionBlock (GroupNorm + single-head
self-attention over spatial positions + residual).

Reference computation (B=32, C=512, H=W=32, N=H*W=1024):
    xn = GroupNorm(32 groups)(x) * gamma + beta
    q/k/v = W{q,k,v} @ xn + b         (per batch, [C, N])
    score = q^T k / sqrt(C)           ([N, N])
    attn  = softmax(score, axis=-1)
    out   = Wo @ (v @ attn^T) + bo    ([C, N])
    y     = out + xn

Sharding: data-parallel over batch across 8 NeuronCores (4 batches each);
weights replicated.

Implementation notes:
- Softmax normalization is deferred to the very end (y = pf * recb + ...),
  which lets the whole attention block collapse algebraically to 4 GEMMs:
    t   = (Wk^T Wq) xn          scoresT = xn^T t   (+ per-key bias term)
    vt  = xn^T (Wo Wv)^T        pf      = vt^T erowT
  The composite weights M2 = Wk^T Wq and Wov = Wo Wv are formed on the
  host. The q-side bias terms are constant along the softmax axis and
  cancel; the k-side term u = Wk^T bq folds into the t copy-out; the
  v/o biases fold into bo2 = bo + Wo bv added with the residual.
- All GEMMs run in fp8 e4m3 with DoubleRow perf mode (2x throughput,
  256-deep contraction per instruction). Weights are pre-scaled x32 on
  the host so they quantize in e4m3's normal range; all scales are
  folded into copy-out constants and the deferred softmax reciprocal.
- GroupNorm statistics, softmax accumulation and the residual stay fp32.
  Measured end-to-end rel l2 error ~6e-3 (gate 2e-2).
"""

import os
import sys

for _p in ("/opt/trn_rl_repo", "/root/.axon_site/_ro/trn_rl_repo"):
    if os.path.isdir(_p) and _p not in sys.path:
        sys.path.insert(0, _p)

import numpy as np
import ml_dtypes

import concourse.bass as bass
import concourse.mybir as mybir
import concourse.tile as tile
from concourse import bacc
from concourse.bass_utils import run_bass_kernel_spmd

# Problem constants (hardcoded per harness contract)
B, C, HH, WW = 32, 512, 32, 32
HW = HH * WW                  # 1024 sequence positions
NCORES = 8
BL = B // NCORES              # batches per core
G = 32                        # groups
GS = C // G                   # channels per group (16)
P = 128                       # partitions
CT = C // P                   # channel chunks (4)
NT = HW // P                  # sequence chunks (8)
NHALF = HW // 512             # 512-wide free-dim halves (2)
EPS = 1e-5
SCALE = float(C) ** -0.5
WS = 32.0                     # fp8 weight pre-scale
C0 = 3.0                      # exp offset: erow = exp(score - C0)
SV = 0.5                      # vt copy-out scale (keeps 32*vt under e4m3 max 240)
ONESV = WS * SV               # den matmul constant; recb = 1/(ONESV*den)
F32 = mybir.dt.float32
BF16 = mybir.dt.bfloat16
FP8 = mybir.dt.float8e4
AF = mybir.ActivationFunctionType
ALU = mybir.AluOpType
DR = mybir.MatmulPerfMode.DoubleRow


def _host_constants():
    # gmat[p, t, g] = 1/(16*HW) if channel (t*128+p) is in group g
    gmat = np.zeros((P, CT, G), dtype=np.float32)
    # hmat[g, t, p] = 1 if channel (t*128+p) is in group g (group -> channel)
    hmat = np.zeros((P, CT, P), dtype=np.float32)
    for t in range(CT):
        for p in range(P):
            g = (t * P + p) // GS
            gmat[p, t, g] = 1.0 / (GS * HW)
            hmat[g, t, p] = 1.0
    return gmat, hmat


def build_module():
    nc = bacc.Bacc("TRN2", target_bir_lowering=False, debug=False)

    x = nc.dram_tensor("x", [BL, C, HW], F32, kind="ExternalInput").ap()
    y = nc.dram_tensor("y", [BL, C, HW], F32, kind="ExternalOutput").ap()
    m2T = nc.dram_tensor("m2T", [C, C], FP8, kind="ExternalInput").ap()
    wovT = nc.dram_tensor("wovT", [C, C], FP8, kind="ExternalInput").ap()
    gamma = nc.dram_tensor("gamma", [C], F32, kind="ExternalInput").ap()
    beta = nc.dram_tensor("beta", [C], F32, kind="ExternalInput").ap()
    uvec = nc.dram_tensor("uvec", [C], F32, kind="ExternalInput").ap()
    bo2 = nc.dram_tensor("bo2", [C], F32, kind="ExternalInput").ap()
    gmat = nc.dram_tensor("gmat", [P, CT, G], F32, kind="ExternalInput").ap()
    hmat = nc.dram_tensor("hmat", [P, CT, P], F32, kind="ExternalInput").ap()

    def pc(v):  # [C] dram -> [P, CT] sbuf layout (channel c = t*128+p)
        return v.rearrange("(t p) -> p t", p=P)

    with tile.TileContext(nc) as tc:
        with (
            tc.tile_pool(name="singles", bufs=1) as singles,
            tc.tile_pool(name="xpool", bufs=3) as xpool,
            tc.tile_pool(name="acts", bufs=2) as acts,
            tc.tile_pool(name="ypool", bufs=2) as ypool,
            tc.tile_pool(name="small", bufs=4) as small,
            tc.tile_pool(name="pbig", bufs=4, space="PSUM") as pbig,
        ):
            # ---- batch 0/1 inputs first: stats chains are the critical path ----
            xs_t = {}

            def emit_load(b):
                xs = xpool.tile([P, CT, HW], F32, tag="xs", name=f"xs{b}")
                xr = x[b].rearrange("(t p) n -> p t n", p=P)
                for t in range(CT):
                    nc.sync.dma_start(out=xs[:, t, :], in_=xr[:, t, :])
                xs_t[b] = xs

            emit_load(0)
            emit_load(1)

            # ---- load constants / weights once ----
            m2_s = singles.tile([P, CT, C], FP8)
            wov_s = singles.tile([P, CT, C], FP8)
            nc.sync.dma_start(out=m2_s, in_=m2T.rearrange("(t p) o -> p t o", p=P))
            nc.sync.dma_start(out=wov_s, in_=wovT.rearrange("(t p) o -> p t o", p=P))
            gmat_s = singles.tile([P, CT, G], F32)
            hmat_s = singles.tile([P, CT, P], F32)
            nc.sync.dma_start(out=gmat_s, in_=gmat)
            nc.sync.dma_start(out=hmat_s, in_=hmat)
            gamma_s = singles.tile([P, CT], F32)
            beta_s = singles.tile([P, CT], F32)
            u_s = singles.tile([P, CT], F32)
            bo2_s = singles.tile([P, CT], F32)
            nc.sync.dma_start(out=gamma_s, in_=pc(gamma))
            nc.sync.dma_start(out=beta_s, in_=pc(beta))
            nc.sync.dma_start(out=u_s, in_=pc(uvec))
            nc.sync.dma_start(out=bo2_s, in_=pc(bo2))
            ones_s = singles.tile([P, 2, P], FP8)
            nc.vector.memset(ones_s, ONESV)
            negc0 = singles.tile([P, 1], F32)
            nc.vector.memset(negc0, -C0)

            # ---- PE warm-up: tiny matmuls so the HAM clock gate opens
            # while batch 0's DMA + stats chain runs ----
            warm = singles.tile([P, 16], BF16)
            nc.vector.memset(warm, 1.0)
            pwarm = pbig.tile([P, 1024], F32, tag="mm")
            for _ in range(280):
                nc.tensor.matmul(pwarm[:16, :16], warm, warm, start=True, stop=True)

            st = {}   # per-batch state: xb8, xbo, t8, erow, vt8

            def emit_stats_xb(b):
                """GroupNorm stats -> A/Bb, then xb8 (vector) + xbo (gpsimd)."""
                xs = xs_t[b]
                stat2 = small.tile([P, CT, 2], F32, tag="stat2", name=f"st{b}")
                for t in range(CT):
                    bnout = small.tile([P, 2, 6], F32, tag="bnout", name=f"bn{b}_{t}")
                    xv = xs[:, t, :].rearrange("p (s f) -> p s f", f=512)
                    for s in range(2):
                        nc.vector.bn_stats(out=bnout[:, s, :], in_=xv[:, s, :])
                    nc.vector.bn_aggr(out=stat2[:, t, :], in_=bnout)
                sq = small.tile([P, CT], F32, tag="sq", name=f"sq{b}")
                nc.vector.tensor_mul(sq, stat2[:, :, 0], stat2[:, :, 0])
                nc.vector.tensor_add(stat2[:, :, 1], stat2[:, :, 1], sq)
                nc.vector.tensor_scalar_mul(stat2, stat2, float(HW))

                # group stats [32, 2] = sum_t gmat[:,t,:].T @ stat2[:,t,:]
                pp = pbig.tile([P, 1024], F32, tag="mm", name=f"pp{b}")
                pg = pp[:G, 0:2]
                for t in range(CT):
                    nc.tensor.matmul(
                        pg,
                        gmat_s[:, t, :],
                        stat2[:, t, :],
                        start=(t == 0),
                        stop=(t == CT - 1),
                    )
                st[b] = {"pp": pp}

            def emit_stats_xb2(b):
                xs = xs_t[b]
                pp = st[b]["pp"]
                pg = pp[:G, 0:2]
                # rstd_g = 1/sqrt(E[x^2]-mean^2+eps);  mrs_g = mean*rstd
                gb = small.tile([P, 2], F32, tag="gb", name=f"gb{b}")
                nc.vector.memset(gb, 0.0)
                pgs = small.tile([G, 2], F32, tag="pgs", name=f"pgs{b}")
                nc.vector.tensor_copy(pgs, pg)
                msq = small.tile([G, 1], F32, tag="msq", name=f"msq{b}")
                nc.vector.tensor_mul(msq, pgs[:, 0:1], pgs[:, 0:1])
                veps = small.tile([G, 1], F32, tag="veps", name=f"veps{b}")
                nc.vector.tensor_scalar(
                    veps, pgs[:, 1:2], msq, EPS, op0=ALU.subtract, op1=ALU.add
                )
                std = small.tile([G, 1], F32, tag="std", name=f"sd{b}")
                nc.scalar.activation(out=std, in_=veps, func=AF.Sqrt)
                nc.vector.reciprocal(gb[:G, 0:1], std)
                nc.vector.tensor_mul(gb[:G, 1:2], pgs[:, 0:1], gb[:G, 0:1])

                # broadcast group -> channel: [p, t, (rstd, mrs)]
                ppc = pp[:, 512 : 512 + 2 * CT].rearrange("p (t k) -> p t k", k=2)
                for t in range(CT):
                    nc.tensor.matmul(
                        ppc[:, t, :], hmat_s[:, t, :], gb, start=True, stop=True
                    )
                # A = gamma * rstd ; Bb = beta - gamma * mean * rstd
                A = acts.tile([P, CT], F32, tag="A", name=f"A{b}")
                Bb = acts.tile([P, CT], F32, tag="Bb", name=f"Bb{b}")
                Bb2 = acts.tile([P, CT], F32, tag="Bb2", name=f"Bb2{b}")
                nc.vector.tensor_mul(A, gamma_s, ppc[:, :, 0])
                nc.vector.tensor_mul(Bb, gamma_s, ppc[:, :, 1])
                nc.vector.tensor_tensor(Bb, beta_s, Bb, op=ALU.subtract)
                nc.vector.tensor_add(Bb2, Bb, bo2_s)

                # xb8 <- fp8(xs*A + Bb) on vector; xbo <- f32 xn + bo2 on
                # gpsimd (residual + folded v/o bias)
                xb8 = acts.tile([P, CT, HW], FP8, tag="xb8", name=f"xb8{b}")
                xbo = acts.tile([P, CT, HW], F32, tag="xbo", name=f"xbo{b}")
                for t in range(CT):
                    nc.scalar.activation(
                        out=xb8[:, t, :],
                        in_=xs[:, t, :],
                        func=AF.Identity,
                        scale=A[:, t : t + 1],
                        bias=Bb[:, t : t + 1],
                    )
                    nc.gpsimd.tensor_scalar(
                        xbo[:, t, :],
                        xs[:, t, :],
                        A[:, t : t + 1],
                        Bb2[:, t : t + 1],
                        op0=ALU.mult,
                        op1=ALU.add,
                    )
                st[b]["xb8"] = xb8
                st[b]["xbo"] = xbo

            def emit_t(b):
                """t = M2 @ xn (+u fold): t8[c, n] fp8. Fills the previous
                batch's exp tail on the tensor engine."""
                xb8 = st[b]["xb8"]
                t8 = acts.tile([P, CT, HW], FP8, tag="t8", name=f"t8{b}")
                for ob in range(CT):
                    pt = pbig.tile([P, 1024], F32, tag="mm", name=f"pt{b}_{ob}")
                    for nh in range(NHALF):
                        for tt in (0, 2):
                            nc.tensor.matmul(
                                pt[:, nh * 512 : (nh + 1) * 512],
                                m2_s[:, tt : tt + 2, ob * P : (ob + 1) * P],
                                xb8[:, tt : tt + 2, nh * 512 : (nh + 1) * 512],
                                start=(tt == 0),
                                stop=(tt == 2),
                                perf_mode=DR,
                            )
                    # t8 = psum/32 + u  (u = Wk^T bq; zero in the common case)
                    if ob % 2 == 0:
                        nc.vector.tensor_scalar(
                            t8[:, ob, :],
                            pt,
                            1.0 / WS,
                            u_s[:, ob : ob + 1],
                            op0=ALU.mult,
                            op1=ALU.add,
                        )
                    else:
                        nc.scalar.activation(
                            out=t8[:, ob, :],
                            in_=pt,
                            func=AF.Identity,
                            scale=1.0 / WS,
                            bias=u_s[:, ob : ob + 1],
                        )
                st[b]["t8"] = t8

            def emit_vt_sc(b):
                """vt chains (front, covers t8-copy latency), then scoresT
                chains + exp per m-block."""
                xb8 = st[b]["xb8"]
                t8 = st[b]["t8"]
                erow = acts.tile([P, NT, HW], FP8, tag="erow", name=f"er{b}")
                vt8 = acts.tile([P, NT, C], FP8, tag="vt8", name=f"vt{b}")

                for j in (0, 2, 4, 6):
                    pv = pbig.tile([P, 1024], F32, tag="mm", name=f"pv{b}_{j}")
                    for jj in (j, j + 1):
                        for tt in (0, 2):
                            nc.tensor.matmul(
                                pv[:, (jj - j) * 512 : (jj - j + 1) * 512],
                                xb8[:, tt : tt + 2, jj * P : (jj + 1) * P],
                                wov_s[:, tt : tt + 2, :],
                                start=(tt == 0),
                                stop=(tt == 2),
                                perf_mode=DR,
                            )
                    # vt8 = SV * psum  (carries WS*SV = 16x true vt)
                    if j % 4 == 0:
                        nc.scalar.activation(
                            out=vt8[:, j : j + 2, :], in_=pv, func=AF.Identity, scale=SV
                        )
                    else:
                        nc.vector.tensor_scalar_mul(vt8[:, j : j + 2, :], pv, SV)

                for i in range(NT):
                    ps = pbig.tile([P, 1024], F32, tag="mm", name=f"ps{b}_{i}")
                    for nh in range(NHALF):
                        for tt in (0, 2):
                            nc.tensor.matmul(
                                ps[:, nh * 512 : (nh + 1) * 512],
                                xb8[:, tt : tt + 2, i * P : (i + 1) * P],
                                t8[:, tt : tt + 2, nh * 512 : (nh + 1) * 512],
                                start=(tt == 0),
                                stop=(tt == 2),
                                perf_mode=DR,
                            )
                    nc.scalar.activation(
                        out=erow[:, i, :],
                        in_=ps,
                        func=AF.Exp,
                        scale=SCALE,
                        bias=negc0,
                    )
                st[b]["erow"] = erow
                st[b]["vt8"] = vt8

            def emit_den_attn(b):
                erow = st[b]["erow"]
                vt8 = st[b]["vt8"]
                xbo = st[b]["xbo"]
                # den (deferred softmax denominator), broadcast over
                # partitions by an all-16s stationary
                pd = pbig.tile([P, 1024], F32, tag="mm", name=f"pd{b}")
                for nh in range(NHALF):
                    for jj in (0, 2, 4, 6):
                        nc.tensor.matmul(
                            pd[:, nh * 512 : (nh + 1) * 512],
                            ones_s,
                            erow[:, jj : jj + 2, nh * 512 : (nh + 1) * 512],
                            start=(jj == 0),
                            stop=(jj == 6),
                            perf_mode=DR,
                        )
                recb = acts.tile([P, HW], F32, tag="recb", name=f"rb{b}")
                nc.vector.reciprocal_approx_fast(out=recb, in_=pd)

                # attention output + residual
                y_s = ypool.tile([P, CT, HW], F32, tag="ys", name=f"ys{b}")
                yr = y[b].rearrange("(t p) n -> p t n", p=P)
                for ob in range(CT):
                    pf = pbig.tile([P, 1024], F32, tag="mm", name=f"pf{b}_{ob}")
                    for nh in range(NHALF):
                        for jj in (0, 2, 4, 6):
                            nc.tensor.matmul(
                                pf[:, nh * 512 : (nh + 1) * 512],
                                vt8[:, jj : jj + 2, ob * P : (ob + 1) * P],
                                erow[:, jj : jj + 2, nh * 512 : (nh + 1) * 512],
                                start=(jj == 0),
                                stop=(jj == 6),
                                perf_mode=DR,
                            )
                    nc.vector.tensor_tensor(y_s[:, ob, :], pf, recb, op=ALU.mult)
                    nc.gpsimd.tensor_tensor(
                        y_s[:, ob, :], y_s[:, ob, :], xbo[:, ob, :], op=ALU.add
                    )
                    nc.sync.dma_start(out=yr[:, ob, :], in_=y_s[:, ob, :])
                del st[b]

            # ---- software-pipelined batch loop: batch b+1's stats/xb8/t
            # fill batch b's exp tail on the tensor engine, so the PE never
            # idles > the HAM re-throttle window ----
            emit_stats_xb(0)
            emit_stats_xb2(0)
            emit_t(0)
            for b in range(BL):
                emit_vt_sc(b)
                if b + 1 < BL:
                    if b + 2 < BL:
                        emit_load(b + 2)
                    emit_stats_xb(b + 1)   # bn_stats + group matmul only
                emit_den_attn(b)
                if b + 1 < BL:
                    emit_stats_xb2(b + 1)  # smalls chain + broadcast + xb8/xbo
                    emit_t(b + 1)

    nc.compile()
    return nc


_NC_CACHE = None


def _get_module():
    global _NC_CACHE
    if _NC_CACHE is None:
        _NC_CACHE = build_module()
    return _NC_CACHE


def make_in_maps(x, gamma, beta, wq, bq, wk, bk, wv, bv, wo, bo):
    x = np.ascontiguousarray(np.asarray(x, dtype=np.float32)).reshape(B, C, HW)
    gmat, hmat = _host_constants()

    f64 = lambda a: np.asarray(a, np.float64)
    wq64, wk64, wv64, wo64 = f64(wq), f64(wk), f64(wv), f64(wo)
    # composite weights (see module docstring); pre-scaled x32 for e4m3
    m2T = np.ascontiguousarray(
        ((wq64.T @ wk64) * WS).astype(np.float32).astype(ml_dtypes.float8_e4m3)
    )
    wovT = np.ascontiguousarray(
        (((wo64 @ wv64).T) * WS).astype(np.float32).astype(ml_dtypes.float8_e4m3)
    )
    uvec = (wk64.T @ f64(bq)).astype(np.float32)
    bo2 = (f64(bo) + wo64 @ f64(bv)).astype(np.float32)

    shared = {
        "m2T": m2T,
        "wovT": wovT,
        "gamma": np.asarray(gamma, np.float32),
        "beta": np.asarray(beta, np.float32),
        "uvec": uvec,
        "bo2": bo2,
        "gmat": gmat,
        "hmat": hmat,
    }
    return [
        {"x": np.ascontiguousarray(x[c * BL : (c + 1) * BL]), **shared}
        for c in range(NCORES)
    ]


def run(inputs, trace=False, **kw):
    nc = _get_module()
    in_maps = make_in_maps(**inputs)
    res = run_bass_kernel_spmd(nc, in_maps, list(range(NCORES)), trace=trace, **kw)
    out = np.concatenate([res.results[c]["y"] for c in range(NCORES)], axis=0)
    return out.reshape(B, C, HH, WW), res


def kernel(**inputs):
    out, _ = run(inputs, trace=False)
    return out


# revision 15
# speedup vs baseline: 1.7936x; 1.0698x over previous
"""Trainium2 Bass kernel for an AttentionBlock (GroupNorm + single-head
self-attention over spatial positions + residual).

Reference computation (B=32, C=512, H=W=32, N=H*W=1024):
    xn = GroupNorm(32 groups)(x) * gamma + beta
    q/k/v = W{q,k,v} @ xn + b         (per batch, [C, N])
    score = q^T k / sqrt(C)           ([N, N])
    attn  = softmax(score, axis=-1)
    out   = Wo @ (v @ attn^T) + bo    ([C, N])
    y     = out + xn

Sharding: data-parallel over batch across 8 NeuronCores (4 batches each);
weights replicated.

Implementation notes:
- Softmax normalization is deferred to the very end (y = pf * recb + ...),
  which lets the whole attention block collapse algebraically to 4 GEMMs:
    t   = (Wk^T Wq) xn          scoresT = xn^T t   (+ per-key bias term)
    vt  = xn^T (Wo Wv)^T        pf      = vt^T erowT
  The composite weights M2 = Wk^T Wq and Wov = Wo Wv are formed on the
  host. The q-side bias terms are constant along the softmax axis and
  cancel; the k-side term u = Wk^T bq folds into the t copy-out; the
  v/o biases fold into bo2 = bo + Wo bv added with the residual.
- All GEMMs run in fp8 e4m3 with DoubleRow perf mode (2x throughput,
  256-deep contraction per instruction). Weights are pre-scaled x32 on
  the host so they quantize in e4m3's normal range; all scales are
  folded into copy-out constants and the deferred softmax reciprocal.
- GroupNorm statistics, softmax accumulation and the residual stay fp32.
  Measured end-to-end rel l2 error ~6e-3 (gate 2e-2).
"""

import os
import sys

for _p in ("/opt/trn_rl_repo", "/root/.axon_site/_ro/trn_rl_repo"):
    if os.path.isdir(_p) and _p not in sys.path:
        sys.path.insert(0, _p)

import numpy as np
import ml_dtypes

import concourse.bass as bass
import concourse.mybir as mybir
import concourse.tile as tile
from concourse import bacc
from concourse.bass_utils import run_bass_kernel_spmd

# Problem constants (hardcoded per harness contract)
B, C, HH, WW = 32, 512, 32, 32
HW = HH * WW                  # 1024 sequence positions
NCORES = 8
BL = B // NCORES              # batches per core
G = 32                        # groups
GS = C // G                   # channels per group (16)
P = 128                       # partitions
CT = C // P                   # channel chunks (4)
NT = HW // P                  # sequence chunks (8)
NHALF = HW // 512             # 512-wide free-dim halves (2)
EPS = 1e-5
SCALE = float(C) ** -0.5
WS = 32.0                     # fp8 weight pre-scale
C0 = 3.0                      # exp offset: erow = exp(score - C0)
SV = 0.5                      # vt copy-out scale (keeps 32*vt under e4m3 max 240)
ONESV = WS * SV               # den matmul constant; recb = 1/(ONESV*den)
F32 = mybir.dt.float32
BF16 = mybir.dt.bfloat16
FP8 = mybir.dt.float8e4
AF = mybir.ActivationFunctionType
ALU = mybir.AluOpType
DR = mybir.MatmulPerfMode.DoubleRow


def _host_constants():
    # gmat[p, t, g] = 1/(16*HW) if channel (t*128+p) is in group g
    gmat = np.zeros((P, CT, G), dtype=np.float32)
    # hmat[g, t, p] = 1 if channel (t*128+p) is in group g (group -> channel)
    hmat = np.zeros((P, CT, P), dtype=np.float32)
    for t in range(CT):
        for p in range(P):
            g = (t * P + p) // GS
            gmat[p, t, g] = 1.0 / (GS * HW)
            hmat[g, t, p] = 1.0
    return gmat, hmat


def build_module():
    nc = bacc.Bacc("TRN2", target_bir_lowering=False, debug=False)

    x = nc.dram_tensor("x", [BL, C, HW], F32, kind="ExternalInput").ap()
    y = nc.dram_tensor("y", [BL, C, HW], F32, kind="ExternalOutput").ap()
    m2T = nc.dram_tensor("m2T", [C, C], FP8, kind="ExternalInput").ap()
    wovT = nc.dram_tensor("wovT", [C, C], FP8, kind="ExternalInput").ap()
    gamma = nc.dram_tensor("gamma", [C], F32, kind="ExternalInput").ap()
    beta = nc.dram_tensor("beta", [C], F32, kind="ExternalInput").ap()
    uvec = nc.dram_tensor("uvec", [C], F32, kind="ExternalInput").ap()
    bo2 = nc.dram_tensor("bo2", [C], F32, kind="ExternalInput").ap()
    gmat = nc.dram_tensor("gmat", [P, CT, G], F32, kind="ExternalInput").ap()
    hmat = nc.dram_tensor("hmat", [P, CT, P], F32, kind="ExternalInput").ap()

    def pc(v):  # [C] dram -> [P, CT] sbuf layout (channel c = t*128+p)
        return v.rearrange("(t p) -> p t", p=P)

    with tile.TileContext(nc) as tc:
        with (
            tc.tile_pool(name="singles", bufs=1) as singles,
            tc.tile_pool(name="xpool", bufs=3) as xpool,
            tc.tile_pool(name="acts", bufs=2) as acts,
            tc.tile_pool(name="ypool", bufs=2) as ypool,
            tc.tile_pool(name="small", bufs=4) as small,
            tc.tile_pool(name="pbig", bufs=4, space="PSUM") as pbig,
        ):
            # ---- batch 0/1 inputs first: stats chains are the critical path ----
            xs_t = {}

            def emit_load(b):
                xs = xpool.tile([P, CT, HW], F32, tag="xs", name=f"xs{b}")
                xr = x[b].rearrange("(t p) n -> p t n", p=P)
                for t in range(CT):
                    nc.sync.dma_start(out=xs[:, t, :], in_=xr[:, t, :])
                xs_t[b] = xs

            emit_load(0)
            emit_load(1)

            # ---- load constants / weights once ----
            m2_s = singles.tile([P, CT, C], FP8)
            wov_s = singles.tile([P, CT, C], FP8)
            nc.sync.dma_start(out=m2_s, in_=m2T.rearrange("(t p) o -> p t o", p=P))
            nc.sync.dma_start(out=wov_s, in_=wovT.rearrange("(t p) o -> p t o", p=P))
            gmat_s = singles.tile([P, CT, G], F32)
            hmat_s = singles.tile([P, CT, P], F32)
            nc.sync.dma_start(out=gmat_s, in_=gmat)
            nc.sync.dma_start(out=hmat_s, in_=hmat)
            gamma_s = singles.tile([P, CT], F32)
            beta_s = singles.tile([P, CT], F32)
            u_s = singles.tile([P, CT], F32)
            bo2_s = singles.tile([P, CT], F32)
            nc.sync.dma_start(out=gamma_s, in_=pc(gamma))
            nc.sync.dma_start(out=beta_s, in_=pc(beta))
            nc.sync.dma_start(out=u_s, in_=pc(uvec))
            nc.sync.dma_start(out=bo2_s, in_=pc(bo2))
            ones_s = singles.tile([P, 2, P], FP8)
            nc.vector.memset(ones_s, ONESV)
            negc0 = singles.tile([P, 1], F32)
            nc.vector.memset(negc0, -C0)

            # ---- PE warm-up: tiny matmuls so the HAM clock gate opens
            # while batch 0's DMA + stats chain runs ----
            warm = singles.tile([P, 16], BF16)
            nc.vector.memset(warm, 1.0)
            pwarm = pbig.tile([P, 1024], F32, tag="mm")
            for _ in range(280):
                nc.tensor.matmul(pwarm[:16, :16], warm, warm, start=True, stop=True)

            st = {}   # per-batch state: xb8, xbo, t8, erow, vt8

            def emit_stats_xb(b):
                """GroupNorm stats -> A/Bb, then xb8 (vector) + xbo (gpsimd)."""
                xs = xs_t[b]
                stat2 = small.tile([P, CT, 2], F32, tag="stat2", name=f"st{b}")
                for t in range(CT):
                    bnout = small.tile([P, 2, 6], F32, tag="bnout", name=f"bn{b}_{t}")
                    xv = xs[:, t, :].rearrange("p (s f) -> p s f", f=512)
                    for s in range(2):
                        nc.vector.bn_stats(out=bnout[:, s, :], in_=xv[:, s, :])
                    nc.vector.bn_aggr(out=stat2[:, t, :], in_=bnout)
                sq = small.tile([P, CT], F32, tag="sq", name=f"sq{b}")
                nc.vector.tensor_mul(sq, stat2[:, :, 0], stat2[:, :, 0])
                nc.vector.tensor_add(stat2[:, :, 1], stat2[:, :, 1], sq)
                nc.vector.tensor_scalar_mul(stat2, stat2, float(HW))

                # group stats [32, 2] = sum_t gmat[:,t,:].T @ stat2[:,t,:]
                pp = pbig.tile([P, 1024], F32, tag="mm", name=f"pp{b}")
                pg = pp[:G, 0:2]
                for t in range(CT):
                    nc.tensor.matmul(
                        pg,
                        gmat_s[:, t, :],
                        stat2[:, t, :],
                        start=(t == 0),
                        stop=(t == CT - 1),
                    )
                st[b] = {"pp": pp}

            def emit_stats_xb2(b):
                xs = xs_t[b]
                pp = st[b]["pp"]
                pg = pp[:G, 0:2]
                # rstd_g = 1/sqrt(E[x^2]-mean^2+eps);  mrs_g = mean*rstd
                gb = small.tile([P, 2], F32, tag="gb", name=f"gb{b}")
                nc.vector.memset(gb, 0.0)
                pgs = small.tile([G, 2], F32, tag="pgs", name=f"pgs{b}")
                nc.vector.tensor_copy(pgs, pg)
                msq = small.tile([G, 1], F32, tag="msq", name=f"msq{b}")
                nc.vector.tensor_mul(msq, pgs[:, 0:1], pgs[:, 0:1])
                veps = small.tile([G, 1], F32, tag="veps", name=f"veps{b}")
                nc.vector.tensor_scalar(
                    veps, pgs[:, 1:2], msq, EPS, op0=ALU.subtract, op1=ALU.add
                )
                # rstd = exp(-0.5 ln v): Ln/Exp/Identity share one act
                # table bucket, so no table reloads (Sqrt would force one)
                lnv = small.tile([G, 1], F32, tag="lnv", name=f"lv{b}")
                nc.scalar.activation(out=lnv, in_=veps, func=AF.Ln)
                nc.scalar.activation(
                    out=gb[:G, 0:1], in_=lnv, func=AF.Exp, scale=-0.5
                )
                nc.vector.tensor_mul(gb[:G, 1:2], pgs[:, 0:1], gb[:G, 0:1])

                # broadcast group -> channel: [p, t, (rstd, mrs)]
                ppc = pp[:, 512 : 512 + 2 * CT].rearrange("p (t k) -> p t k", k=2)
                for t in range(CT):
                    nc.tensor.matmul(
                        ppc[:, t, :], hmat_s[:, t, :], gb, start=True, stop=True
                    )
                # A = gamma * rstd ; Bb = beta - gamma * mean * rstd
                A = acts.tile([P, CT], F32, tag="A", name=f"A{b}")
                Bb = acts.tile([P, CT], F32, tag="Bb", name=f"Bb{b}")
                Bb2 = acts.tile([P, CT], F32, tag="Bb2", name=f"Bb2{b}")
                nc.vector.tensor_mul(A, gamma_s, ppc[:, :, 0])
                nc.vector.tensor_mul(Bb, gamma_s, ppc[:, :, 1])
                nc.vector.tensor_tensor(Bb, beta_s, Bb, op=ALU.subtract)
                nc.vector.tensor_add(Bb2, Bb, bo2_s)

                st[b]["A"] = A
                st[b]["Bb"] = Bb
                st[b]["Bb2"] = Bb2

            def emit_xb(b):
                """xb8 <- fp8(xs*A + Bb) split scalar/vector; xbo <- f32
                xn + bo2 on gpsimd (residual + folded v/o bias)."""
                xs = xs_t[b]
                A, Bb, Bb2 = st[b]["A"], st[b]["Bb"], st[b]["Bb2"]
                xb8 = acts.tile([P, CT, HW], FP8, tag="xb8", name=f"xb8{b}")
                xbo = acts.tile([P, CT, HW], F32, tag="xbo", name=f"xbo{b}")
                for t in range(CT):
                    if t % 2 == 0:
                        nc.scalar.activation(
                            out=xb8[:, t, :],
                            in_=xs[:, t, :],
                            func=AF.Identity,
                            scale=A[:, t : t + 1],
                            bias=Bb[:, t : t + 1],
                        )
                    else:
                        nc.vector.tensor_scalar(
                            xb8[:, t, :],
                            xs[:, t, :],
                            A[:, t : t + 1],
                            Bb[:, t : t + 1],
                            op0=ALU.mult,
                            op1=ALU.add,
                        )
                    nc.gpsimd.tensor_scalar(
                        xbo[:, t, :],
                        xs[:, t, :],
                        A[:, t : t + 1],
                        Bb2[:, t : t + 1],
                        op0=ALU.mult,
                        op1=ALU.add,
                    )
                st[b]["xb8"] = xb8
                st[b]["xbo"] = xbo

            def emit_t(b):
                """t = M2 @ xn (+u fold): t8[c, n] fp8. Fills the previous
                batch's exp tail on the tensor engine."""
                xb8 = st[b]["xb8"]
                t8 = acts.tile([P, CT, HW], FP8, tag="t8", name=f"t8{b}")
                for ob in range(CT):
                    pt = pbig.tile([P, 1024], F32, tag="mm", name=f"pt{b}_{ob}")
                    for nh in range(NHALF):
                        for tt in (0, 2):
                            nc.tensor.matmul(
                                pt[:, nh * 512 : (nh + 1) * 512],
                                m2_s[:, tt : tt + 2, ob * P : (ob + 1) * P],
                                xb8[:, tt : tt + 2, nh * 512 : (nh + 1) * 512],
                                start=(tt == 0),
                                stop=(tt == 2),
                                perf_mode=DR,
                            )
                    # t8 = psum/32 + u  (u = Wk^T bq; zero in the common case)
                    if ob % 2 == 0:
                        nc.vector.tensor_scalar(
                            t8[:, ob, :],
                            pt,
                            1.0 / WS,
                            u_s[:, ob : ob + 1],
                            op0=ALU.mult,
                            op1=ALU.add,
                        )
                    else:
                        nc.scalar.activation(
                            out=t8[:, ob, :],
                            in_=pt,
                            func=AF.Identity,
                            scale=1.0 / WS,
                            bias=u_s[:, ob : ob + 1],
                        )
                st[b]["t8"] = t8

            def emit_vt_sc(b):
                """vt chains (front, covers t8-copy latency), then scoresT
                chains + exp per m-block."""
                xb8 = st[b]["xb8"]
                t8 = st[b]["t8"]
                erow = acts.tile([P, NT, HW], FP8, tag="erow", name=f"er{b}")
                vt8 = acts.tile([P, NT, C], FP8, tag="vt8", name=f"vt{b}")

                for j in (0, 2, 4, 6):
                    pv = pbig.tile([P, 1024], F32, tag="mm", name=f"pv{b}_{j}")
                    for jj in (j, j + 1):
                        for tt in (0, 2):
                            nc.tensor.matmul(
                                pv[:, (jj - j) * 512 : (jj - j + 1) * 512],
                                xb8[:, tt : tt + 2, jj * P : (jj + 1) * P],
                                wov_s[:, tt : tt + 2, :],
                                start=(tt == 0),
                                stop=(tt == 2),
                                perf_mode=DR,
                            )
                    # vt8 = SV * psum  (carries WS*SV = 16x true vt)
                    if j % 4 == 0:
                        nc.scalar.activation(
                            out=vt8[:, j : j + 2, :], in_=pv, func=AF.Identity, scale=SV
                        )
                    else:
                        nc.vector.tensor_scalar_mul(vt8[:, j : j + 2, :], pv, SV)

                for i in range(NT):
                    ps = pbig.tile([P, 1024], F32, tag="mm", name=f"ps{b}_{i}")
                    for nh in range(NHALF):
                        for tt in (0, 2):
                            nc.tensor.matmul(
                                ps[:, nh * 512 : (nh + 1) * 512],
                                xb8[:, tt : tt + 2, i * P : (i + 1) * P],
                                t8[:, tt : tt + 2, nh * 512 : (nh + 1) * 512],
                                start=(tt == 0),
                                stop=(tt == 2),
                                perf_mode=DR,
                            )
                    nc.scalar.activation(
                        out=erow[:, i, :],
                        in_=ps,
                        func=AF.Exp,
                        scale=SCALE,
                        bias=negc0,
                    )
                    # next-batch prep interleaved into this phase so the
                    # stats chain latency hides under sc work and the
                    # tensor stream never gaps past the HAM window
                    if b + 1 < BL:
                        if i == 1:
                            if b + 2 < BL:
                                emit_load(b + 2)
                            emit_stats_xb(b + 1)
                        elif i == 5:
                            emit_stats_xb2(b + 1)
                        elif i == 7:
                            emit_xb(b + 1)
                st[b]["erow"] = erow
                st[b]["vt8"] = vt8

            def emit_den_attn(b):
                erow = st[b]["erow"]
                vt8 = st[b]["vt8"]
                xbo = st[b]["xbo"]
                # den (deferred softmax denominator), broadcast over
                # partitions by an all-16s stationary
                pd = pbig.tile([P, 1024], F32, tag="mm", name=f"pd{b}")
                for nh in range(NHALF):
                    for jj in (0, 2, 4, 6):
                        nc.tensor.matmul(
                            pd[:, nh * 512 : (nh + 1) * 512],
                            ones_s,
                            erow[:, jj : jj + 2, nh * 512 : (nh + 1) * 512],
                            start=(jj == 0),
                            stop=(jj == 6),
                            perf_mode=DR,
                        )
                recb = acts.tile([P, HW], F32, tag="recb", name=f"rb{b}")
                nc.vector.reciprocal_approx_fast(out=recb, in_=pd)

                # attention output + residual
                y_s = ypool.tile([P, CT, HW], F32, tag="ys", name=f"ys{b}")
                yr = y[b].rearrange("(t p) n -> p t n", p=P)
                for ob in range(CT):
                    pf = pbig.tile([P, 1024], F32, tag="mm", name=f"pf{b}_{ob}")
                    for nh in range(NHALF):
                        for jj in (0, 2, 4, 6):
                            nc.tensor.matmul(
                                pf[:, nh * 512 : (nh + 1) * 512],
                                vt8[:, jj : jj + 2, ob * P : (ob + 1) * P],
                                erow[:, jj : jj + 2, nh * 512 : (nh + 1) * 512],
                                start=(jj == 0),
                                stop=(jj == 6),
                                perf_mode=DR,
                            )
                    nc.vector.tensor_tensor(y_s[:, ob, :], pf, recb, op=ALU.mult)
                    nc.gpsimd.tensor_tensor(
                        y_s[:, ob, :], y_s[:, ob, :], xbo[:, ob, :], op=ALU.add
                    )
                    nc.sync.dma_start(out=yr[:, ob, :], in_=y_s[:, ob, :])
                del st[b]

            # ---- software-pipelined batch loop: batch b+1's stats/xb8/t
            # fill batch b's exp tail on the tensor engine, so the PE never
            # idles > the HAM re-throttle window ----
            emit_stats_xb(0)
            emit_stats_xb2(0)
            emit_xb(0)
            emit_t(0)
            for b in range(BL):
                emit_vt_sc(b)
                emit_den_attn(b)
                if b + 1 < BL:
                    emit_t(b + 1)

    nc.compile()
    return nc


_NC_CACHE = None


def _get_module():
    global _NC_CACHE
    if _NC_CACHE is None:
        _NC_CACHE = build_module()
    return _NC_CACHE


def make_in_maps(x, gamma, beta, wq, bq, wk, bk, wv, bv, wo, bo):
    x = np.ascontiguousarray(np.asarray(x, dtype=np.float32)).reshape(B, C, HW)
    gmat, hmat = _host_constants()

    f64 = lambda a: np.asarray(a, np.float64)
    wq64, wk64, wv64, wo64 = f64(wq), f64(wk), f64(wv), f64(wo)
    # composite weights (see module docstring); pre-scaled x32 for e4m3
    m2T = np.ascontiguousarray(
        ((wq64.T @ wk64) * WS).astype(np.float32).astype(ml_dtypes.float8_e4m3)
    )
    wovT = np.ascontiguousarray(
        (((wo64 @ wv64).T) * WS).astype(np.float32).astype(ml_dtypes.float8_e4m3)
    )
    uvec = (wk64.T @ f64(bq)).astype(np.float32)
    bo2 = (f64(bo) + wo64 @ f64(bv)).astype(np.float32)

    shared = {
        "m2T": m2T,
        "wovT": wovT,
        "gamma": np.asarray(gamma, np.float32),
        "beta": np.asarray(beta, np.float32),
        "uvec": uvec,
        "bo2": bo2,
        "gmat": gmat,
        "hmat": hmat,
    }
    return [
        {"x": np.ascontiguousarray(x[c * BL : (c + 1) * BL]), **shared}
        for c in range(NCORES)
    ]


def run(inputs, trace=False, **kw):
    nc = _get_module()
    in_maps = make_in_maps(**inputs)
    res = run_bass_kernel_spmd(nc, in_maps, list(range(NCORES)), trace=trace, **kw)
    out = np.concatenate([res.results[c]["y"] for c in range(NCORES)], axis=0)
    return out.reshape(B, C, HH, WW), res


def kernel(**inputs):
    out, _ = run(inputs, trace=False)
    return out
